# revision 1
# baseline (speedup 1.0000x reference)
"""HGSA (hypergraph attention) layer on 8 trn2 NeuronCores.

Reference math:
  feat_src = (feat @ fc_w)  ->  [N, h, d]
  e(p)     = leaky_relu(s[src_p, h] + t[edge_p, h]);  s = feat_src . attn_src, t = edge_feat . attn_edge
  attn     = per-hyperedge softmax over incident pairs
  hyper[e] = seg_sum(attn * feat_src[src])            [E, h, d]
  rst      = H @ hyper                                [N, h*d]

Identities used (everything becomes dense matmuls over H; no dense exp/gather):
  - softmax max-subtraction cancels exactly; logits are O(1) so plain exp is safe.
  - exp(lrelu(x)), x = s+t, splits by sign r = [x>0]:
        w = r*u*v + (1-r)*u2*v2,  u=exp(s), v=exp(t), u2=exp(.2s), v2=exp(.2t)
  - with G1 = H .* r and Fu = [feat_src_h * u | u] (33 cols), Fu2 likewise:
        masked sums = v .* (Fu^T @ G1) + v2 .* (Fu2^T @ H - Fu2^T @ G1)
  - sign tile trick (exact): S = sign(H*(t+C) + (s-C)) with C > max|s|,|t| gives
        S = +1 iff (H=1 and s+t>0) else -1 (ties -> 0, which is also exact for w).
        Fu^T@G1 = .5*(Fu^T@S) + .5*colsum(Fu).
    fp16 carries (t+C) with ~4e-3 abs error -> sign flips only within |s+t|<4e-3
    where both branches of w agree to <0.4%; net output error ~1e-5.

Sharding: node rows split 2500/core over 8 cores; per-edge aggregates
AllReduce'd; dissemination uses fp16 H^T tiles (H is 0/1 -> exact) with hi/lo
fp16 split of the hyperedge features for ~fp26 precision.

Layout note: SBUF/PSUM partition bases must be 0/32/64, so the per-head
stationary matrix is padded to 97 rows: [Fu (33) | zeros (31) | Fu2 (33)] and
extractions use bases 0 and 64.
"""

from contextlib import ExitStack

import numpy as np

import concourse.bass as bass
import concourse.mybir as mybir
import concourse.tile as tile
from concourse import bacc
from concourse.bass_utils import run_bass_kernel_spmd
from concourse.masks import make_identity

F32 = mybir.dt.float32
F32R = mybir.dt.float32r
F16 = mybir.dt.float16

N_NODES, N_EDGES = 20000, 2000
IN_FEATS, NUM_HEADS, OUT_FEATS, EDGE_DIM = 128, 4, 32, 64
NEG_SLOPE = 0.2
CORES = 8
NPC = N_NODES // CORES          # 2500 nodes per core
EBLK = 500                      # phase-A edge block (one PSUM bank of f32)
NBLK = N_EDGES // EBLK          # 4 edge blocks
NT = (NPC + 127) // 128         # 20 node tiles per core (19x128 + 68)
EPAD = 2048                     # padded edges for H^T xbar loads
NPAD = 2560                     # padded nodes per core
ET = EPAD // 128                # 16 e-tiles in dissemination
C_OFF = 8.0                     # sign-trick offset, > max|s|, max|t|


def _nt(k):
    n0 = k * 128
    return n0, min(128, NPC - n0)


def r32(ap):
    return ap


def build_kernel(nc):
    feat_d = nc.dram_tensor("feat", [NPC, IN_FEATS], F32, kind="ExternalInput").ap()
    h_d = nc.dram_tensor("H", [NPC, N_EDGES], F32, kind="ExternalInput").ap()
    ef_d = nc.dram_tensor("edge_feat", [N_EDGES, EDGE_DIM], F32, kind="ExternalInput").ap()
    fcw_d = nc.dram_tensor("fc_w", [IN_FEATS, IN_FEATS], F32, kind="ExternalInput").ap()
    asrc_d = nc.dram_tensor("attn_src", [1, NUM_HEADS * OUT_FEATS], F32, kind="ExternalInput").ap()
    aedgeT_d = nc.dram_tensor("attn_edgeT", [EDGE_DIM, NUM_HEADS], F32, kind="ExternalInput").ap()
    rst_d = nc.dram_tensor("rst", [NPC, NUM_HEADS * OUT_FEATS], F32, kind="ExternalOutput").ap()

    with tile.TileContext(nc) as tc, ExitStack() as ctx:
        consts = ctx.enter_context(tc.tile_pool(name="consts", bufs=1))
        prep = ctx.enter_context(tc.tile_pool(name="prep", bufs=2))
        persist = ctx.enter_context(tc.tile_pool(name="persist", bufs=1))
        hpool = ctx.enter_context(tc.tile_pool(name="hpool", bufs=4))
        work = ctx.enter_context(tc.tile_pool(name="work", bufs=2))
        psum = ctx.enter_context(tc.tile_pool(name="psum", bufs=2, space="PSUM"))
        psA = ctx.enter_context(tc.tile_pool(name="psA", bufs=1, space="PSUM"))
        dram = ctx.enter_context(tc.tile_pool(name="dram", bufs=1, space="DRAM"))

        ident = consts.tile([128, 128], F32)
        make_identity(nc, ident)
        ones_row = consts.tile([1, 128], F32)
        nc.gpsimd.memset(ones_row[:], 1.0)
        ones_col = consts.tile([128, 1], F32)
        nc.gpsimd.memset(ones_col[:], 1.0)
        ones_col16 = consts.tile([128, 1], F16)
        nc.gpsimd.memset(ones_col16[:], 1.0)
        zpad = consts.tile([128, EPAD], F16)
        nc.gpsimd.memset(zpad[:], 0.0)

        # ---------------- stage 0a: small params ----------------
        fcw = persist.tile([128, 128], F32)
        nc.sync.dma_start(fcw[:], fcw_d[:, :])
        attn_s = persist.tile([1, 128], F32)
        nc.sync.dma_start(attn_s[:], asrc_d[:, :])
        attn_eT = persist.tile([64, NUM_HEADS], F32)
        nc.sync.dma_start(attn_eT[:], aedgeT_d[:, :])

        # w_s[k, h] = sum_d fc_w[k, h*32+d] * attn_src[h*32+d]   [128, 4]
        asb_ps = psum.tile([128, 128], F32, tag="ps")
        nc.tensor.matmul(asb_ps[:, :], r32(ones_row[:, :]), r32(attn_s[:, :]),
                         start=True, stop=True)
        asb = prep.tile([128, 128], F32, tag="asb")
        nc.vector.tensor_copy(asb[:, :], asb_ps[:, :])
        fwa = prep.tile([128, 128], F32, tag="fwa")
        nc.vector.tensor_tensor(fwa[:, :], fcw[:, :], asb[:, :], mybir.AluOpType.mult)
        w_s = persist.tile([128, NUM_HEADS], F32)
        for h in range(NUM_HEADS):
            nc.vector.tensor_reduce(w_s[:, h:h + 1], fwa[:, h * 32:(h + 1) * 32],
                                    mybir.AxisListType.X, mybir.AluOpType.add)

        # ---------------- stage 0b: node projections ----------------
        # fa[k]: [128, 4*97], head block = [Fu (33) | zeros (31) | Fu2 (33)]
        fs_tiles, u_tiles, sc_tiles, fa_tiles, fa2_tiles = [], [], [], [], []
        for k in range(NT):
            n0, nn = _nt(k)
            ft = prep.tile([128, 128], F32, tag="ft")
            nc.sync.dma_start(ft[:nn, :], feat_d[n0:n0 + nn, :])
            ftT_ps = psum.tile([128, 128], F32, tag="ps")
            nc.tensor.transpose(ftT_ps[:, :nn], ft[:nn, :], ident[:nn, :nn])
            ftT = prep.tile([128, 128], F32, tag="ftT")
            nc.vector.tensor_copy(ftT[:, :nn], ftT_ps[:, :nn])
            fs_ps = psum.tile([128, 128], F32, tag="ps")
            nc.tensor.matmul(fs_ps[:nn, :], r32(ftT[:, :nn]), r32(fcw[:, :]),
                             start=True, stop=True)
            fs = prep.tile([128, 128], F32, tag="fs")
            nc.vector.tensor_copy(fs[:nn, :], fs_ps[:nn, :])
            fs_tiles.append(fs)
            # s[n, h] then u = exp(s), u2 = exp(.2 s); sc = s - C
            s_ps = psum.tile([128, NUM_HEADS], F32, tag="ps")
            nc.tensor.matmul(s_ps[:nn, :], r32(ftT[:, :nn]), r32(w_s[:, :]),
                             start=True, stop=True)
            u_t = persist.tile([128, 2 * NUM_HEADS], F32, tag=f"u{k}")
            nc.scalar.activation(u_t[:nn, 0:NUM_HEADS], s_ps[:nn, :],
                                 mybir.ActivationFunctionType.Exp)
            nc.scalar.activation(u_t[:nn, NUM_HEADS:], s_ps[:nn, :],
                                 mybir.ActivationFunctionType.Exp, scale=NEG_SLOPE)
            u_tiles.append(u_t)
            sc = persist.tile([128, NUM_HEADS], F32, tag=f"sc{k}")
            nc.vector.tensor_scalar_add(sc[:nn, :], s_ps[:nn, :], -C_OFF)
            sc_tiles.append(sc)

            fa = persist.tile([128, NUM_HEADS * 97], F16, tag=f"fa{k}")
            nc.vector.memset(fa[:], 0.0)
            for h in range(NUM_HEADS):
                u_c = u_t[:nn, h:h + 1]
                u2_c = u_t[:nn, NUM_HEADS + h:NUM_HEADS + h + 1]
                b0 = h * 97
                nc.vector.tensor_scalar_mul(fa[:nn, b0:b0 + 32],
                                            fs[:nn, h * 32:(h + 1) * 32], u_c)
                nc.vector.tensor_copy(fa[:nn, b0 + 32:b0 + 33], u_c)
                nc.scalar.activation(fa[:nn, b0 + 64:b0 + 96],
                                     fs[:nn, h * 32:(h + 1) * 32],
                                     mybir.ActivationFunctionType.Copy, scale=u2_c)
                nc.scalar.copy(fa[:nn, b0 + 96:b0 + 97], u2_c)
            fa_tiles.append(fa)
            # fa2[k][p]: [128, 97] = [Fu2_{2p} (33) | zeros | Fu2_{2p+1} (33)]
            fa2_pair = []
            for p in range(2):
                fa2 = persist.tile([128, 97], F16, tag=f"fa2_{k}_{p}")
                nc.vector.memset(fa2[:], 0.0)
                h0, h1 = 2 * p, 2 * p + 1
                nc.vector.tensor_copy(fa2[:nn, 0:33], fa[:nn, h0 * 97 + 64:h0 * 97 + 97])
                nc.vector.tensor_copy(fa2[:nn, 64:97], fa[:nn, h1 * 97 + 64:h1 * 97 + 97])
                fa2_pair.append(fa2)
            fa2_tiles.append(fa2_pair)

        # ---------------- stage 0c: edge side ----------------
        efT = persist.tile([64, N_EDGES], F32)
        for j in range((N_EDGES + 127) // 128):
            e0 = j * 128
            ee = min(128, N_EDGES - e0)
            ef = prep.tile([128, 64], F32, tag="ef")
            nc.sync.dma_start(ef[:ee, :], ef_d[e0:e0 + ee, :])
            ef_ps = psum.tile([64, 128], F32, tag="ps")
            nc.tensor.transpose(ef_ps[:, :ee], ef[:ee, :], ident[:ee, :ee])
            nc.vector.tensor_copy(efT[:, e0:e0 + ee], ef_ps[:, :ee])
        # transient per-(h,b) t rows -> tcb fp16 bcast tiles; v rows rebuilt later
        def t_row_ps(h, b):
            sl = slice(b * EBLK, (b + 1) * EBLK)
            t_ps = psum.tile([1, EBLK], F32, tag="ps", name="t_ps")
            nc.tensor.matmul(t_ps[:, :], r32(attn_eT[:, h:h + 1]), r32(efT[:, sl]),
                             start=True, stop=True)
            return t_ps

        tcb = [[None] * NBLK for _ in range(NUM_HEADS)]
        for h in range(NUM_HEADS):
            for b in range(NBLK):
                t_ps = t_row_ps(h, b)
                tC_row = prep.tile([1, EBLK], F32, tag="tC_row")
                nc.vector.tensor_scalar_add(tC_row[:, :], t_ps[:, :], C_OFF)
                ps = psum.tile([128, EBLK], F32, tag="ps")
                nc.tensor.matmul(ps[:, :], r32(ones_row[:, :]), r32(tC_row[:, :]),
                                 start=True, stop=True)
                t16 = persist.tile([128, EBLK], F16, tag=f"tcb{h}_{b}")
                nc.vector.tensor_copy(t16[:, :], ps[:, :])
                tcb[h][b] = t16

        # H fp16 staging DRAM (padded) for phase-C transposed reads
        h16_dram = dram.tile([NPAD, EPAD], F16)
        nc.sync.dma_start(h16_dram[NPC:NPAD, :], zpad[: NPAD - NPC, :])

        # ---------------- colsum (needs only fa tiles) ----------------
        csU = persist.tile([33, NUM_HEADS], F32)
        csU2 = persist.tile([33, NUM_HEADS], F32)
        for h in range(NUM_HEADS):
            ps_c = psA.tile([97, 1], F32, tag="psh0", name="ps_c")
            for k in range(NT):
                n0, nn = _nt(k)
                nc.tensor.matmul(ps_c[:, :], r32(fa_tiles[k][:nn, h * 97:(h + 1) * 97]),
                                 ones_col16[:nn, :], start=(k == 0), stop=(k == NT - 1))
            nc.vector.tensor_copy(csU[:, h:h + 1], ps_c[0:33, :])
            nc.vector.tensor_copy(csU2[:, h:h + 1], ps_c[64:97, :])
        half_csU = persist.tile([33, NUM_HEADS], F32)
        half_csU2 = persist.tile([33, NUM_HEADS], F32)
        nc.vector.tensor_scalar_mul(half_csU[:, :], csU[:, :], 0.5)
        nc.vector.tensor_scalar_mul(half_csU2[:, :], csU2[:, :], 0.5)

        # ---------------- phase A ----------------
        aggU = [persist.tile([33, N_EDGES], F32, tag=f"aggU{h}", name=f"aggU{h}") for h in range(NUM_HEADS)]

        for b in range(NBLK):
            e0 = b * EBLK
            ps_g = [psA.tile([97, EBLK], F32, tag=f"psg{h}", name=f"psg{h}") for h in range(NUM_HEADS)]
            ps_h = [psA.tile([97, EBLK], F32, tag=f"psh{p}", name=f"psh{p}") for p in range(2)]
            for k in range(NT):
                n0, nn = _nt(k)
                ht = hpool.tile([128, EBLK], F32, tag="ht")
                nc.sync.dma_start(ht[:nn, :], h_d[n0:n0 + nn, e0:e0 + EBLK])
                h16 = hpool.tile([128, EBLK], F16, tag="h16")
                nc.gpsimd.tensor_copy(h16[:nn, :], ht[:nn, :])
                nc.sync.dma_start(h16_dram[n0:n0 + nn, e0:e0 + EBLK], h16[:nn, :])
                if b == 0:
                    nc.sync.dma_start(h16_dram[n0:n0 + nn, N_EDGES:EPAD],
                                      zpad[:nn, N_EDGES:EPAD])
                first, last = (k == 0), (k == NT - 1)
                fa = fa_tiles[k]
                for h in range(NUM_HEADS):
                    htc = work.tile([128, EBLK], F16, tag="htc")
                    nc.vector.tensor_tensor(htc[:nn, :], h16[:nn, :], tcb[h][b][:nn, :],
                                            mybir.AluOpType.mult)
                    sgn = work.tile([128, EBLK], F16, tag="sgn")
                    nc.scalar.activation(sgn[:nn, :], htc[:nn, :],
                                         mybir.ActivationFunctionType.Sign,
                                         bias=sc_tiles[k][:nn, h:h + 1])
                    nc.tensor.matmul(ps_g[h][:, :], r32(fa[:nn, h * 97:(h + 1) * 97]),
                                     r32(sgn[:nn, :]), start=first, stop=last)
                for p in range(2):
                    nc.tensor.matmul(ps_h[p][:, :], fa2_tiles[k][p][:nn, :],
                                     h16[:nn, :], start=first, stop=last)
            # fused combine for this block, reading PSUM directly:
            #   A1u  = .5*psg[0:33]  + .5*csU ;  A1u2 = .5*psg[64:97] + .5*csU2
            #   aggU = v .* A1u + v2 .* (A2 - A1u2)
            for h in range(NUM_HEADS):
                p, hh = divmod(h, 2)
                sl = slice(e0, e0 + EBLK)
                t_ps = t_row_ps(h, b)
                v_row = prep.tile([1, 2 * EBLK], F32, tag="v_row")
                nc.scalar.activation(v_row[:, 0:EBLK], t_ps[:, :],
                                     mybir.ActivationFunctionType.Exp)
                nc.scalar.activation(v_row[:, EBLK:], t_ps[:, :],
                                     mybir.ActivationFunctionType.Exp, scale=NEG_SLOPE)
                vb_ps = psum.tile([33, EBLK], F32, tag="ps")
                nc.tensor.matmul(vb_ps[:, :], r32(ones_row[:, 0:33]),
                                 r32(v_row[:, 0:EBLK]), start=True, stop=True)
                v2b_ps = psum.tile([33, EBLK], F32, tag="ps")
                nc.tensor.matmul(v2b_ps[:, :], r32(ones_row[:, 0:33]),
                                 r32(v_row[:, EBLK:]), start=True, stop=True)
                a1u = work.tile([33, EBLK], F32, tag="a1u")
                nc.vector.tensor_scalar(a1u[:, :], ps_g[h][0:33, :], 0.5,
                                        half_csU[:, h:h + 1], mybir.AluOpType.mult,
                                        mybir.AluOpType.add)
                a1u2 = work.tile([33, EBLK], F32, tag="a1u2")
                nc.vector.tensor_scalar(a1u2[:, :], ps_g[h][64:97, :], 0.5,
                                        half_csU2[:, h:h + 1], mybir.AluOpType.mult,
                                        mybir.AluOpType.add)
                d2 = work.tile([33, EBLK], F32, tag="d2")
                a2v = ps_h[p][0:33, :] if hh == 0 else ps_h[p][64:97, :]
                nc.vector.tensor_tensor(d2[:, :], a2v, a1u2[:, :],
                                        mybir.AluOpType.subtract)
                nc.vector.tensor_tensor(d2[:, :], d2[:, :], v2b_ps[:, :],
                                        mybir.AluOpType.mult)
                nc.vector.tensor_tensor(a1u[:, :], a1u[:, :], vb_ps[:, :],
                                        mybir.AluOpType.mult)
                nc.vector.tensor_tensor(aggU[h][:, sl], a1u[:, :], d2[:, :],
                                        mybir.AluOpType.add)

        # ---------------- collective ----------------
        cc_in = dram.tile([NUM_HEADS, 33, N_EDGES], F32)
        cc_out = dram.tile([NUM_HEADS, 33, N_EDGES], F32)
        for h in range(NUM_HEADS):
            nc.gpsimd.dma_start(cc_in[h, :, :], aggU[h][:, :])
        nc.gpsimd.collective_compute(
            "AllReduce",
            mybir.AluOpType.add,
            replica_groups=[list(range(CORES))],
            ins=[cc_in.opt()],
            outs=[cc_out.opt()],
        )
        for h in range(NUM_HEADS):
            nc.gpsimd.dma_start(aggU[h][:, :], cc_out[h, :, :])

        # ---------------- normalize -> hyper hi/lo fp16 [128e, 128hd] x 16 ----------------
        hyper_hi = [persist.tile([128, 128], F16, tag=f"hhi{et}", name=f"hhi{et}") for et in range(ET)]
        hyper_lo = [persist.tile([128, 128], F16, tag=f"hlo{et}", name=f"hlo{et}") for et in range(ET)]
        for et in range(ET):
            e0 = et * 128
            ee = max(0, min(128, N_EDGES - e0))
            hyp = work.tile([128, 128], F32, tag="hyp")
            if ee < 128:
                nc.vector.memset(hyp[:], 0.0)
            for h in range(NUM_HEADS):
                if ee == 0:
                    continue
                tps = psum.tile([128, 33], F32, tag="ps")
                nc.tensor.transpose(tps[:ee, :], aggU[h][:, e0:e0 + ee],
                                    ident[0:33, 0:33])
                at = work.tile([128, 33], F32, tag="at")
                nc.vector.tensor_copy(at[:ee, :], tps[:ee, :])
                den = work.tile([128, 1], F32, tag="den")
                nc.vector.tensor_scalar_add(den[:ee, :], at[:ee, 32:33], 1e-9)
                rec = work.tile([128, 1], F32, tag="rec")
                nc.vector.reciprocal(rec[:ee, :], den[:ee, :])
                nc.vector.tensor_scalar_mul(hyp[:ee, h * 32:(h + 1) * 32],
                                            at[:ee, 0:32], rec[:ee, :])
            hi32 = work.tile([128, 128], F32, tag="hi32")
            nc.vector.tensor_copy(hyper_hi[et][:, :], hyp[:, :])
            nc.vector.tensor_copy(hi32[:, :], hyper_hi[et][:, :])
            nc.vector.tensor_tensor(hi32[:, :], hyp[:, :], hi32[:, :],
                                    mybir.AluOpType.subtract)
            nc.vector.tensor_copy(hyper_lo[et][:, :], hi32[:, :])

        # ---------------- phase C: rst = H @ hyper ----------------
        NCH = NPAD // 512
        for nch in range(NCH):
            h0 = nch * 512
            rps = [psA.tile([128, 128], F32, tag=f"psg{j}", name=f"psr{j}") for j in range(4)]
            for et in range(ET):
                htt = hpool.tile([128, 512], F16, tag="htt")
                nc.sync.dma_start_transpose(htt[:, :],
                                            h16_dram[h0:h0 + 512, et * 128:(et + 1) * 128])
                for j in range(4):
                    nc.tensor.matmul(rps[j][:, :], htt[:, j * 128:(j + 1) * 128],
                                     hyper_hi[et][:, :], start=(et == 0), stop=False)
                    nc.tensor.matmul(rps[j][:, :], htt[:, j * 128:(j + 1) * 128],
                                     hyper_lo[et][:, :], start=False, stop=(et == ET - 1))
            for j in range(4):
                n0 = h0 + j * 128
                if n0 >= NPC:
                    break
                nn = min(128, NPC - n0)
                rt = work.tile([128, 128], F32, tag="rt")
                nc.vector.tensor_copy(rt[:nn, :], rps[j][:nn, :])
                nc.sync.dma_start(rst_d[n0:n0 + nn, :], rt[:nn, :])

    return nc


PROFILE = False
LAST_RUN_NS = None

_CACHE = {}


def _get_nc():
    if "nc" not in _CACHE:
        nc = bacc.Bacc("TRN2", target_bir_lowering=False, debug=False,
                       enable_asserts=False, num_devices=CORES)
        build_kernel(nc)
        nc.compile()
        _CACHE["nc"] = nc
    return _CACHE["nc"]


def kernel(feat, edge_feat, H, fc_w, attn_src, attn_edge, src_idx=None, edge_idx=None,
           **extra):
    feat = np.ascontiguousarray(np.asarray(feat, np.float32))
    edge_feat = np.ascontiguousarray(np.asarray(edge_feat, np.float32))
    H = np.ascontiguousarray(np.asarray(H, np.float32))
    fc_w = np.ascontiguousarray(np.asarray(fc_w, np.float32))
    attn_src_f = np.ascontiguousarray(
        np.asarray(attn_src, np.float32).reshape(1, NUM_HEADS * OUT_FEATS))
    attn_edgeT = np.ascontiguousarray(
        np.asarray(attn_edge, np.float32).reshape(NUM_HEADS, EDGE_DIM).T)

    nc = _get_nc()
    in_maps = []
    for c in range(CORES):
        r0 = c * NPC
        in_maps.append({
            "feat": np.ascontiguousarray(feat[r0:r0 + NPC]),
            "H": np.ascontiguousarray(H[r0:r0 + NPC]),
            "edge_feat": edge_feat,
            "fc_w": fc_w,
            "attn_src": attn_src_f,
            "attn_edgeT": attn_edgeT,
        })
    import time as _time
    _t0 = _time.time()
    res = run_bass_kernel_spmd(nc, in_maps, list(range(CORES)))
    global LAST_RUN_NS
    LAST_RUN_NS = int((_time.time() - _t0) * 1e9)
    out = np.concatenate([res.results[c]["rst"] for c in range(CORES)], axis=0)
    return out



# revision 7
# speedup vs baseline: 3.2336x; 3.2336x over previous
"""HGSA (hypergraph attention) layer on 8 trn2 NeuronCores.

Reference math:
  feat_src = (feat @ fc_w)  ->  [N, h, d]
  e(p)     = leaky_relu(s[src_p, h] + t[edge_p, h]);  s = feat_src . attn_src, t = edge_feat . attn_edge
  attn     = per-hyperedge softmax over incident pairs
  hyper[e] = seg_sum(attn * feat_src[src])            [E, h, d]
  rst      = H @ hyper                                [N, h*d]

Identities used (everything becomes dense matmuls over H; no dense exp/gather):
  - softmax max-subtraction cancels exactly; logits are O(1) so plain exp is safe.
  - exp(lrelu(x)), x = s+t, splits by sign r = [x>0]:
        w = r*u*v + (1-r)*u2*v2,  u=exp(s), v=exp(t), u2=exp(.2s), v2=exp(.2t)
  - with G1 = H .* r and Fu = [feat_src_h * u | u] (33 cols), Fu2 likewise:
        masked sums = v .* (Fu^T @ G1) + v2 .* (Fu2^T @ H - Fu2^T @ G1)
  - sign tile trick (exact): S = sign(H*(t+C) + (s-C)) with C > max|s|,|t| gives
        S = +1 iff (H=1 and s+t>0) else -1 (ties -> 0, which is also exact for w).
        Fu^T@G1 = .5*(Fu^T@S) + .5*colsum(Fu).

I/O diet (the axon tunnel, not the device, is the bottleneck):
  - H enters bit-packed (uint8, 8 edges/byte) and is unpacked on-device with
    vector shift/and into an fp16 0/1 tile. The edge axis is globally
    permuted into "bitplane" order e=8j+k -> k*250+j so the unpack writes
    contiguous 250-col blocks; t rows are permuted to match on the host and
    the permutation cancels everywhere else (it never leaves the edge axis).
  - feat enters fp16 and is loaded via transposed DMA (no on-chip transpose).
  - t = edge_feat . attn_edge and w_s = fc_w . attn_src are computed on the
    host (tiny) so edge_feat/attn_* never cross the wire.
  - rst leaves as fp16.

Sharding: node rows split 2500/core (padded to 2560) over 8 cores; per-edge
aggregates AllReduce'd; dissemination uses fp16 H^T tiles (H is 0/1 -> exact)
with hi/lo fp16 split of the hyperedge features for ~fp26 precision.

Layout note: SBUF/PSUM partition bases must be 0/32/64, so the per-head
stationary matrix is padded to 97 rows: [Fu (33) | zeros (31) | Fu2 (33)] and
extractions use bases 0 and 64.
"""

from contextlib import ExitStack

import numpy as np

import concourse.bass as bass
import concourse.mybir as mybir
import concourse.tile as tile
from concourse import bacc
from concourse.bass_utils import run_bass_kernel_spmd
from concourse.masks import make_identity

F32 = mybir.dt.float32
F16 = mybir.dt.float16
U8 = mybir.dt.uint8

N_NODES, N_EDGES = 20000, 2000
IN_FEATS, NUM_HEADS, OUT_FEATS, EDGE_DIM = 128, 4, 32, 64
NEG_SLOPE = 0.2
CORES = 8
NPC = N_NODES // CORES          # 2500 nodes per core
NPAD = 2560                     # padded nodes per core (20 full 128-tiles)
NT = NPAD // 128                # 20 node tiles per core
PBYTES = N_EDGES // 8           # 250 packed bytes per node row
EBLK = 500                      # phase-A edge block = 2 bitplanes of 250
NBLK = N_EDGES // EBLK          # 4 edge blocks
EPAD = 2048                     # padded edges for H^T xbar loads
ET = EPAD // 128                # 16 e-tiles in dissemination
C_OFF = 8.0                     # sign-trick offset, > max|s|, max|t|


def build_kernel(nc):
    feat_d = nc.dram_tensor("feat16", [NPAD, IN_FEATS], F16, kind="ExternalInput").ap()
    hp_d = nc.dram_tensor("hpack", [NPAD, PBYTES], U8, kind="ExternalInput").ap()
    trow_d = nc.dram_tensor("trow", [1, NUM_HEADS * N_EDGES], F32, kind="ExternalInput").ap()
    fcw_d = nc.dram_tensor("fcw16", [IN_FEATS, IN_FEATS], F16, kind="ExternalInput").ap()
    ws_d = nc.dram_tensor("ws16", [IN_FEATS, NUM_HEADS], F16, kind="ExternalInput").ap()
    rst_d = nc.dram_tensor("rst", [NPC, NUM_HEADS * OUT_FEATS], F16, kind="ExternalOutput").ap()

    with tile.TileContext(nc) as tc, ExitStack() as ctx:
        consts = ctx.enter_context(tc.tile_pool(name="consts", bufs=1))
        prep = ctx.enter_context(tc.tile_pool(name="prep", bufs=2))
        persist = ctx.enter_context(tc.tile_pool(name="persist", bufs=1))
        hpool = ctx.enter_context(tc.tile_pool(name="hpool", bufs=4))
        work = ctx.enter_context(tc.tile_pool(name="work", bufs=2))
        psum = ctx.enter_context(tc.tile_pool(name="psum", bufs=2, space="PSUM"))
        psA = ctx.enter_context(tc.tile_pool(name="psA", bufs=1, space="PSUM"))
        dram = ctx.enter_context(tc.tile_pool(name="dram", bufs=1, space="DRAM"))

        ident = consts.tile([128, 128], F32)
        make_identity(nc, ident)
        ones_row = consts.tile([1, 128], F32)
        nc.gpsimd.memset(ones_row[:], 1.0)
        ones_col16 = consts.tile([128, 1], F16)
        nc.gpsimd.memset(ones_col16[:], 1.0)
        zpadc = consts.tile([128, EPAD - N_EDGES], F16)
        nc.gpsimd.memset(zpadc[:], 0.0)

        # ---------------- stage 0a: small params ----------------
        fcw = persist.tile([128, 128], F16)
        nc.sync.dma_start(fcw[:], fcw_d[:, :])
        wst = persist.tile([128, NUM_HEADS], F16)
        nc.sync.dma_start(wst[:], ws_d[:, :])
        tsb = persist.tile([1, NUM_HEADS * N_EDGES], F32)
        nc.sync.dma_start(tsb[:], trow_d[:, :])

        # ---------------- stage 0b: node projections ----------------
        # fa[k]: [128, 4*97], head block = [Fu (33) | zeros (31) | Fu2 (33)]
        fa_tiles, sc_tiles, fa2_tiles = [], [], []
        for k in range(NT):
            n0 = k * 128
            ftT = prep.tile([128, 128], F16, tag="ftT")
            nc.sync.dma_start_transpose(ftT[:, :], feat_d[n0:n0 + 128, :])
            fs_ps = psum.tile([128, 128], F32, tag="ps")
            nc.tensor.matmul(fs_ps[:, :], ftT[:, :], fcw[:, :], start=True, stop=True)
            fs = prep.tile([128, 128], F32, tag="fs")
            nc.vector.tensor_copy(fs[:, :], fs_ps[:, :])
            # s[n, h] then u = exp(s), u2 = exp(.2 s); sc = s - C
            s_ps = psum.tile([128, NUM_HEADS], F32, tag="ps")
            nc.tensor.matmul(s_ps[:, :], ftT[:, :], wst[:, :], start=True, stop=True)
            u_t = prep.tile([128, 2 * NUM_HEADS], F32, tag="u")
            nc.scalar.activation(u_t[:, 0:NUM_HEADS], s_ps[:, :],
                                 mybir.ActivationFunctionType.Exp)
            nc.scalar.activation(u_t[:, NUM_HEADS:], s_ps[:, :],
                                 mybir.ActivationFunctionType.Exp, scale=NEG_SLOPE)
            sc = persist.tile([128, NUM_HEADS], F32, tag=f"sc{k}")
            nc.vector.tensor_scalar_add(sc[:, :], s_ps[:, :], -C_OFF)
            sc_tiles.append(sc)

            fa = persist.tile([128, NUM_HEADS * 97], F16, tag=f"fa{k}")
            nc.vector.memset(fa[:], 0.0)
            for h in range(NUM_HEADS):
                u_c = u_t[:, h:h + 1]
                u2_c = u_t[:, NUM_HEADS + h:NUM_HEADS + h + 1]
                b0 = h * 97
                nc.vector.tensor_scalar_mul(fa[:, b0:b0 + 32],
                                            fs[:, h * 32:(h + 1) * 32], u_c)
                nc.vector.tensor_copy(fa[:, b0 + 32:b0 + 33], u_c)
                nc.scalar.activation(fa[:, b0 + 64:b0 + 96],
                                     fs[:, h * 32:(h + 1) * 32],
                                     mybir.ActivationFunctionType.Copy, scale=u2_c)
                nc.scalar.copy(fa[:, b0 + 96:b0 + 97], u2_c)
            fa_tiles.append(fa)
            # fa2[k][p]: [128, 97] = [Fu2_{2p} (33) | zeros | Fu2_{2p+1} (33)]
            fa2_pair = []
            for p in range(2):
                fa2 = persist.tile([128, 97], F16, tag=f"fa2_{k}_{p}")
                nc.vector.memset(fa2[:], 0.0)
                h0, h1 = 2 * p, 2 * p + 1
                nc.vector.tensor_copy(fa2[:, 0:33], fa[:, h0 * 97 + 64:h0 * 97 + 97])
                nc.vector.tensor_copy(fa2[:, 64:97], fa[:, h1 * 97 + 64:h1 * 97 + 97])
                fa2_pair.append(fa2)
            fa2_tiles.append(fa2_pair)

        # ---------------- edge side: tcb fp16 bcast tiles of (t+C) ----------------
        tcb = [[None] * NBLK for _ in range(NUM_HEADS)]
        for h in range(NUM_HEADS):
            for b in range(NBLK):
                sl = slice(h * N_EDGES + b * EBLK, h * N_EDGES + (b + 1) * EBLK)
                tC_row = prep.tile([1, EBLK], F32, tag="tC_row")
                nc.vector.tensor_scalar_add(tC_row[:, :], tsb[0:1, sl], C_OFF)
                ps = psum.tile([128, EBLK], F32, tag="ps")
                nc.tensor.matmul(ps[:, :], ones_row[:, :], tC_row[:, :],
                                 start=True, stop=True)
                t16 = persist.tile([128, EBLK], F16, tag=f"tcb{h}_{b}")
                nc.vector.tensor_copy(t16[:, :], ps[:, :])
                tcb[h][b] = t16

        # H fp16 staging DRAM (padded) for phase-C transposed reads
        h16_dram = dram.tile([NPAD, EPAD], F16)

        # ---------------- colsum (needs only fa tiles) ----------------
        csU = persist.tile([33, NUM_HEADS], F32)
        csU2 = persist.tile([33, NUM_HEADS], F32)
        for h in range(NUM_HEADS):
            ps_c = psA.tile([97, 1], F32, tag="psh0", name="ps_c")
            for k in range(NT):
                nc.tensor.matmul(ps_c[:, :], fa_tiles[k][:, h * 97:(h + 1) * 97],
                                 ones_col16[:, :], start=(k == 0), stop=(k == NT - 1))
            nc.vector.tensor_copy(csU[:, h:h + 1], ps_c[0:33, :])
            nc.vector.tensor_copy(csU2[:, h:h + 1], ps_c[64:97, :])
        half_csU = persist.tile([33, NUM_HEADS], F32)
        half_csU2 = persist.tile([33, NUM_HEADS], F32)
        nc.vector.tensor_scalar_mul(half_csU[:, :], csU[:, :], 0.5)
        nc.vector.tensor_scalar_mul(half_csU2[:, :], csU2[:, :], 0.5)

        # ---------------- phase A ----------------
        aggU = [persist.tile([33, N_EDGES], F32, tag=f"aggU{h}", name=f"aggU{h}")
                for h in range(NUM_HEADS)]

        for b in range(NBLK):
            e0 = b * EBLK
            ps_g = [psA.tile([97, EBLK], F32, tag=f"psg{h}", name=f"psg{h}")
                    for h in range(NUM_HEADS)]
            ps_h = [psA.tile([97, EBLK], F32, tag=f"psh{p}", name=f"psh{p}")
                    for p in range(2)]
            for k in range(NT):
                n0 = k * 128
                pt = hpool.tile([128, PBYTES], U8, tag="pt")
                nc.sync.dma_start(pt[:, :], hp_d[n0:n0 + 128, :])
                h16 = hpool.tile([128, EBLK], F16, tag="h16")
                for half in range(2):
                    plane = 2 * b + half
                    pu = hpool.tile([128, PBYTES], U8, tag="pu")
                    nc.vector.tensor_scalar(pu[:, :], pt[:, :], 7 - plane, 1,
                                            mybir.AluOpType.logical_shift_right,
                                            mybir.AluOpType.bitwise_and)
                    nc.vector.tensor_copy(h16[:, half * PBYTES:(half + 1) * PBYTES],
                                          pu[:, :])
                nc.sync.dma_start(h16_dram[n0:n0 + 128, e0:e0 + EBLK], h16[:, :])
                if b == 0:
                    nc.sync.dma_start(h16_dram[n0:n0 + 128, N_EDGES:EPAD], zpadc[:, :])
                first, last = (k == 0), (k == NT - 1)
                fa = fa_tiles[k]
                for h in range(NUM_HEADS):
                    htc = work.tile([128, EBLK], F16, tag="htc")
                    nc.vector.tensor_tensor(htc[:, :], h16[:, :], tcb[h][b][:, :],
                                            mybir.AluOpType.mult)
                    sgn = work.tile([128, EBLK], F16, tag="sgn")
                    nc.scalar.activation(sgn[:, :], htc[:, :],
                                         mybir.ActivationFunctionType.Sign,
                                         bias=sc_tiles[k][:, h:h + 1])
                    nc.tensor.matmul(ps_g[h][:, :], fa[:, h * 97:(h + 1) * 97],
                                     sgn[:, :], start=first, stop=last)
                for p in range(2):
                    nc.tensor.matmul(ps_h[p][:, :], fa2_tiles[k][p][:, :],
                                     h16[:, :], start=first, stop=last)
            # fused combine for this block, reading PSUM directly:
            #   A1u  = .5*psg[0:33]  + .5*csU ;  A1u2 = .5*psg[64:97] + .5*csU2
            #   aggU = v .* A1u + v2 .* (A2 - A1u2)
            for h in range(NUM_HEADS):
                p, hh = divmod(h, 2)
                sl = slice(e0, e0 + EBLK)
                tsl = slice(h * N_EDGES + e0, h * N_EDGES + e0 + EBLK)
                v_row = prep.tile([1, 2 * EBLK], F32, tag="v_row")
                nc.scalar.activation(v_row[:, 0:EBLK], tsb[0:1, tsl],
                                     mybir.ActivationFunctionType.Exp)
                nc.scalar.activation(v_row[:, EBLK:], tsb[0:1, tsl],
                                     mybir.ActivationFunctionType.Exp, scale=NEG_SLOPE)
                vb_ps = psum.tile([33, EBLK], F32, tag="ps")
                nc.tensor.matmul(vb_ps[:, :], ones_row[:, 0:33], v_row[:, 0:EBLK],
                                 start=True, stop=True)
                v2b_ps = psum.tile([33, EBLK], F32, tag="ps")
                nc.tensor.matmul(v2b_ps[:, :], ones_row[:, 0:33], v_row[:, EBLK:],
                                 start=True, stop=True)
                a1u = work.tile([33, EBLK], F32, tag="a1u")
                nc.vector.tensor_scalar(a1u[:, :], ps_g[h][0:33, :], 0.5,
                                        half_csU[:, h:h + 1], mybir.AluOpType.mult,
                                        mybir.AluOpType.add)
                a1u2 = work.tile([33, EBLK], F32, tag="a1u2")
                nc.vector.tensor_scalar(a1u2[:, :], ps_g[h][64:97, :], 0.5,
                                        half_csU2[:, h:h + 1], mybir.AluOpType.mult,
                                        mybir.AluOpType.add)
                d2 = work.tile([33, EBLK], F32, tag="d2")
                a2v = ps_h[p][0:33, :] if hh == 0 else ps_h[p][64:97, :]
                nc.vector.tensor_tensor(d2[:, :], a2v, a1u2[:, :],
                                        mybir.AluOpType.subtract)
                nc.vector.tensor_tensor(d2[:, :], d2[:, :], v2b_ps[:, :],
                                        mybir.AluOpType.mult)
                nc.vector.tensor_tensor(a1u[:, :], a1u[:, :], vb_ps[:, :],
                                        mybir.AluOpType.mult)
                nc.vector.tensor_tensor(aggU[h][:, sl], a1u[:, :], d2[:, :],
                                        mybir.AluOpType.add)

        # ---------------- collective ----------------
        cc_in = dram.tile([NUM_HEADS, 33, N_EDGES], F32)
        cc_out = dram.tile([NUM_HEADS, 33, N_EDGES], F32)
        for h in range(NUM_HEADS):
            nc.gpsimd.dma_start(cc_in[h, :, :], aggU[h][:, :])
        nc.gpsimd.collective_compute(
            "AllReduce",
            mybir.AluOpType.add,
            replica_groups=[list(range(CORES))],
            ins=[cc_in.opt()],
            outs=[cc_out.opt()],
        )
        for h in range(NUM_HEADS):
            nc.gpsimd.dma_start(aggU[h][:, :], cc_out[h, :, :])

        # ---------------- normalize -> hyper hi/lo fp16 [128e, 128hd] x 16 ----------------
        hyper_hi = [persist.tile([128, 128], F16, tag=f"hhi{et}", name=f"hhi{et}")
                    for et in range(ET)]
        hyper_lo = [persist.tile([128, 128], F16, tag=f"hlo{et}", name=f"hlo{et}")
                    for et in range(ET)]
        for et in range(ET):
            e0 = et * 128
            ee = max(0, min(128, N_EDGES - e0))
            hyp = work.tile([128, 128], F32, tag="hyp")
            if ee < 128:
                nc.vector.memset(hyp[:], 0.0)
            for h in range(NUM_HEADS):
                if ee == 0:
                    continue
                tps = psum.tile([128, 33], F32, tag="ps")
                nc.tensor.transpose(tps[:ee, :], aggU[h][:, e0:e0 + ee],
                                    ident[0:33, 0:33])
                at = work.tile([128, 33], F32, tag="at")
                nc.vector.tensor_copy(at[:ee, :], tps[:ee, :])
                den = work.tile([128, 1], F32, tag="den")
                nc.vector.tensor_scalar_add(den[:ee, :], at[:ee, 32:33], 1e-9)
                rec = work.tile([128, 1], F32, tag="rec")
                nc.vector.reciprocal(rec[:ee, :], den[:ee, :])
                nc.vector.tensor_scalar_mul(hyp[:ee, h * 32:(h + 1) * 32],
                                            at[:ee, 0:32], rec[:ee, :])
            hi32 = work.tile([128, 128], F32, tag="hi32")
            nc.vector.tensor_copy(hyper_hi[et][:, :], hyp[:, :])
            nc.vector.tensor_copy(hi32[:, :], hyper_hi[et][:, :])
            nc.vector.tensor_tensor(hi32[:, :], hyp[:, :], hi32[:, :],
                                    mybir.AluOpType.subtract)
            nc.vector.tensor_copy(hyper_lo[et][:, :], hi32[:, :])

        # ---------------- phase C: rst = H @ hyper ----------------
        NCH = NPAD // 512
        for nch in range(NCH):
            h0 = nch * 512
            rps = [psA.tile([128, 128], F32, tag=f"psg{j}", name=f"psr{j}")
                   for j in range(4)]
            for et in range(ET):
                htt = hpool.tile([128, 512], F16, tag="htt")
                nc.sync.dma_start_transpose(htt[:, :],
                                            h16_dram[h0:h0 + 512, et * 128:(et + 1) * 128])
                for j in range(4):
                    nc.tensor.matmul(rps[j][:, :], htt[:, j * 128:(j + 1) * 128],
                                     hyper_hi[et][:, :], start=(et == 0), stop=False)
                    nc.tensor.matmul(rps[j][:, :], htt[:, j * 128:(j + 1) * 128],
                                     hyper_lo[et][:, :], start=False, stop=(et == ET - 1))
            for j in range(4):
                n0 = h0 + j * 128
                if n0 >= NPC:
                    break
                nn = min(128, NPC - n0)
                rt = work.tile([128, 128], F16, tag="rt")
                nc.vector.tensor_copy(rt[:nn, :], rps[j][:nn, :])
                nc.sync.dma_start(rst_d[n0:n0 + nn, :], rt[:nn, :])

    return nc


PROFILE = False
LAST_RUN_NS = None

_CACHE = {}


def _get_nc():
    if "nc" not in _CACHE:
        nc = bacc.Bacc("TRN2", target_bir_lowering=False, debug=False,
                       enable_asserts=False, num_devices=CORES)
        build_kernel(nc)
        nc.compile()
        _CACHE["nc"] = nc
    return _CACHE["nc"]


def kernel(feat, edge_feat, H, fc_w, attn_src, attn_edge, src_idx=None, edge_idx=None,
           **extra):
    feat = np.asarray(feat, np.float32)
    edge_feat = np.asarray(edge_feat, np.float32)
    fc_w = np.asarray(fc_w, np.float32)
    a_src = np.asarray(attn_src, np.float32).reshape(NUM_HEADS, OUT_FEATS)
    a_edge = np.asarray(attn_edge, np.float32).reshape(NUM_HEADS, EDGE_DIM)

    # bit-packed incidence (big-endian bit order, matching np.packbits)
    if src_idx is not None and edge_idx is not None:
        si = np.asarray(src_idx, np.int64)
        ei = np.asarray(edge_idx, np.int64)
        hp = np.zeros((N_NODES, PBYTES), np.uint8)
        np.bitwise_or.at(hp, (si, ei >> 3),
                         np.right_shift(128, ei & 7).astype(np.uint8))
    else:
        hp = np.packbits(np.asarray(H, np.float32) != 0, axis=1)

    # t rows in bitplane-permuted edge order: col k*250+j <- edge 8j+k
    t = edge_feat @ a_edge.T                                   # [E, h]
    t_perm = np.ascontiguousarray(
        t.reshape(PBYTES, 8, NUM_HEADS).transpose(2, 1, 0).reshape(NUM_HEADS, N_EDGES)
    ).astype(np.float32)
    ws = (fc_w.reshape(IN_FEATS, NUM_HEADS, OUT_FEATS) * a_src[None]).sum(-1)

    feat16 = np.zeros((CORES, NPAD, IN_FEATS), np.float16)
    feat16[:, :NPC] = feat.reshape(CORES, NPC, IN_FEATS)
    hp8 = np.zeros((CORES, NPAD, PBYTES), np.uint8)
    hp8[:, :NPC] = hp.reshape(CORES, NPC, PBYTES)
    fcw16 = fc_w.astype(np.float16)
    ws16 = ws.astype(np.float16)

    nc = _get_nc()
    t_perm = t_perm.reshape(1, NUM_HEADS * N_EDGES)
    in_maps = [{
        "feat16": feat16[c],
        "hpack": hp8[c],
        "trow": t_perm,
        "fcw16": fcw16,
        "ws16": ws16,
    } for c in range(CORES)]
    import time as _time
    _t0 = _time.time()
    res = run_bass_kernel_spmd(nc, in_maps, list(range(CORES)))
    global LAST_RUN_NS
    LAST_RUN_NS = int((_time.time() - _t0) * 1e9)
    out = np.concatenate([res.results[c]["rst"] for c in range(CORES)], axis=0)
    return out.astype(np.float32)


# revision 12
# speedup vs baseline: 3.5032x; 1.0834x over previous
"""HGSA (hypergraph attention) layer on 8 trn2 NeuronCores.

Reference math:
  feat_src = (feat @ fc_w)  ->  [N, h, d]
  e(p)     = leaky_relu(s[src_p, h] + t[edge_p, h]);  s = feat_src . attn_src, t = edge_feat . attn_edge
  attn     = per-hyperedge softmax over incident pairs
  hyper[e] = seg_sum(attn * feat_src[src])            [E, h, d]
  rst      = H @ hyper                                [N, h*d]

Identities used (everything becomes dense matmuls over H; no dense exp/gather):
  - softmax max-subtraction cancels exactly; logits are O(1) so plain exp is safe.
  - exp(lrelu(x)), x = s+t, splits by sign r = [x>0]:
        w = r*u*v + (1-r)*u2*v2,  u=exp(s), v=exp(t), u2=exp(.2s), v2=exp(.2t)
  - with G1 = H .* r and Fu = [feat_src_h * u | u] (33 cols), Fu2 likewise:
        masked sums = v .* (Fu^T @ G1) + v2 .* (Fu2^T @ H - Fu2^T @ G1)
  - sign tile trick (exact): S = sign(H*(t+C) + (s-C)) with C > max|s|,|t| gives
        S = +1 iff (H=1 and s+t>0) else -1 (ties -> 0, which is also exact for w).
        Fu^T@G1 = .5*(Fu^T@S) + .5*colsum(Fu).

I/O diet (the axon tunnel, not the device, is the bottleneck):
  - H enters bit-packed (uint8, 8 edges/byte) and is unpacked on-device with
    vector shift/and into an fp16 0/1 tile. The edge axis is globally
    permuted into "bitplane" order e=8j+k -> k*250+j so the unpack writes
    contiguous 250-col blocks; t rows are permuted to match on the host and
    the permutation cancels everywhere else (it never leaves the edge axis).
  - feat enters fp16 and is loaded via transposed DMA (no on-chip transpose).
  - t = edge_feat . attn_edge and w_s = fc_w . attn_src are computed on the
    host (tiny) so edge_feat/attn_* never cross the wire.
  - rst leaves as fp16.

Sharding: node rows split 2500/core (padded to 2560) over 8 cores; per-edge
aggregates AllReduce'd; dissemination uses fp16 H^T tiles (H is 0/1 -> exact)
with hi/lo fp16 split of the hyperedge features for ~fp26 precision.

Layout note: SBUF/PSUM partition bases must be 0/32/64, so the per-head
stationary matrix is padded to 97 rows: [Fu (33) | zeros (31) | Fu2 (33)] and
extractions use bases 0 and 64.
"""

from contextlib import ExitStack

import numpy as np

import concourse.bass as bass
import concourse.mybir as mybir
import concourse.tile as tile
from concourse import bacc
from concourse.bass_utils import run_bass_kernel_spmd
from concourse.masks import make_identity

F32 = mybir.dt.float32
F16 = mybir.dt.float16
U8 = mybir.dt.uint8

N_NODES, N_EDGES = 20000, 2000
IN_FEATS, NUM_HEADS, OUT_FEATS, EDGE_DIM = 128, 4, 32, 64
NEG_SLOPE = 0.2
CORES = 8
NPC = N_NODES // CORES          # 2500 nodes per core
NPAD = 2560                     # padded nodes per core (20 full 128-tiles)
NT = NPAD // 128                # 20 node tiles per core
PBYTES = N_EDGES // 8           # 250 packed bytes per node row
EBLK = 500                      # phase-A edge block = 2 bitplanes of 250
NBLK = N_EDGES // EBLK          # 4 edge blocks
EPAD = 2048                     # padded edges for H^T xbar loads
ET = EPAD // 128                # 16 e-tiles in dissemination
C_OFF = 8.0                     # sign-trick offset, > max|s|, max|t|


def build_kernel(nc):
    feat_d = nc.dram_tensor("feat16", [NPAD, IN_FEATS], F16, kind="ExternalInput").ap()
    hp_d = nc.dram_tensor("hpack", [NPAD, PBYTES], U8, kind="ExternalInput").ap()
    trow_d = nc.dram_tensor("trow", [1, NUM_HEADS * N_EDGES], F32, kind="ExternalInput").ap()
    fcw_d = nc.dram_tensor("fcw16", [IN_FEATS, IN_FEATS], F16, kind="ExternalInput").ap()
    ws_d = nc.dram_tensor("ws16", [IN_FEATS, NUM_HEADS], F16, kind="ExternalInput").ap()
    rstT_d = nc.dram_tensor("rstT", [NUM_HEADS * OUT_FEATS, NPC], F16, kind="ExternalOutput").ap()

    with tile.TileContext(nc) as tc, ExitStack() as ctx:
        consts = ctx.enter_context(tc.tile_pool(name="consts", bufs=1))
        prep = ctx.enter_context(tc.tile_pool(name="prep", bufs=2))
        persist = ctx.enter_context(tc.tile_pool(name="persist", bufs=1))
        hpool = ctx.enter_context(tc.tile_pool(name="hpool", bufs=4))
        work = ctx.enter_context(tc.tile_pool(name="work", bufs=2))
        psum = ctx.enter_context(tc.tile_pool(name="psum", bufs=2, space="PSUM"))
        psA = ctx.enter_context(tc.tile_pool(name="psA", bufs=1, space="PSUM"))
        dram = ctx.enter_context(tc.tile_pool(name="dram", bufs=1, space="DRAM"))

        ident = consts.tile([128, 128], F32)
        make_identity(nc, ident)
        ones_row = consts.tile([1, 128], F32)
        nc.gpsimd.memset(ones_row[:], 1.0)
        ones_col16 = consts.tile([128, 1], F16)
        nc.gpsimd.memset(ones_col16[:], 1.0)
        zpadc = consts.tile([128, EPAD - N_EDGES], F16)
        nc.gpsimd.memset(zpadc[:], 0.0)

        # ---------------- stage 0a: small params ----------------
        fcw = persist.tile([128, 128], F16)
        nc.sync.dma_start(fcw[:], fcw_d[:, :])
        wst = persist.tile([128, NUM_HEADS], F16)
        nc.sync.dma_start(wst[:], ws_d[:, :])
        tsb = persist.tile([1, NUM_HEADS * N_EDGES], F32)
        nc.sync.dma_start(tsb[:], trow_d[:, :])

        # ---------------- stage 0b: node projections ----------------
        # fa[k]: [128, 4*97], head block = [Fu (33) | zeros (31) | Fu2 (33)]
        fa_tiles, sc_tiles, fa2_tiles, pt_tiles = [], [], [], []
        for k in range(NT):
            n0 = k * 128
            pt = persist.tile([128, PBYTES], U8, tag=f"pt{k}")
            nc.sync.dma_start(pt[:, :], hp_d[n0:n0 + 128, :])
            pt_tiles.append(pt)
            ftT = prep.tile([128, 128], F16, tag="ftT")
            nc.sync.dma_start_transpose(ftT[:, :], feat_d[n0:n0 + 128, :])
            fs_ps = psum.tile([128, 128], F32, tag="ps")
            nc.tensor.matmul(fs_ps[:, :], ftT[:, :], fcw[:, :], start=True, stop=True)
            fs = prep.tile([128, 128], F32, tag="fs")
            nc.vector.tensor_copy(fs[:, :], fs_ps[:, :])
            # s[n, h] then u = exp(s), u2 = exp(.2 s); sc = s - C
            s_ps = psum.tile([128, NUM_HEADS], F32, tag="ps")
            nc.tensor.matmul(s_ps[:, :], ftT[:, :], wst[:, :], start=True, stop=True)
            u_t = prep.tile([128, 2 * NUM_HEADS], F32, tag="u")
            nc.scalar.activation(u_t[:, 0:NUM_HEADS], s_ps[:, :],
                                 mybir.ActivationFunctionType.Exp)
            nc.scalar.activation(u_t[:, NUM_HEADS:], s_ps[:, :],
                                 mybir.ActivationFunctionType.Exp, scale=NEG_SLOPE)
            sc = persist.tile([128, NUM_HEADS], F32, tag=f"sc{k}")
            nc.vector.tensor_scalar_add(sc[:, :], s_ps[:, :], -C_OFF)
            sc_tiles.append(sc)

            fa = persist.tile([128, NUM_HEADS * 97], F16, tag=f"fa{k}")
            nc.vector.memset(fa[:], 0.0)
            for h in range(NUM_HEADS):
                u_c = u_t[:, h:h + 1]
                u2_c = u_t[:, NUM_HEADS + h:NUM_HEADS + h + 1]
                b0 = h * 97
                nc.vector.tensor_scalar_mul(fa[:, b0:b0 + 32],
                                            fs[:, h * 32:(h + 1) * 32], u_c)
                nc.vector.tensor_copy(fa[:, b0 + 32:b0 + 33], u_c)
                nc.scalar.activation(fa[:, b0 + 64:b0 + 96],
                                     fs[:, h * 32:(h + 1) * 32],
                                     mybir.ActivationFunctionType.Copy, scale=u2_c)
                nc.scalar.copy(fa[:, b0 + 96:b0 + 97], u2_c)
            fa_tiles.append(fa)
            # fa2[k][p]: [128, 97] = [Fu2_{2p} (33) | zeros | Fu2_{2p+1} (33)]
            fa2_pair = []
            for p in range(2):
                fa2 = persist.tile([128, 97], F16, tag=f"fa2_{k}_{p}")
                nc.vector.memset(fa2[:], 0.0)
                h0, h1 = 2 * p, 2 * p + 1
                nc.vector.tensor_copy(fa2[:, 0:33], fa[:, h0 * 97 + 64:h0 * 97 + 97])
                nc.vector.tensor_copy(fa2[:, 64:97], fa[:, h1 * 97 + 64:h1 * 97 + 97])
                fa2_pair.append(fa2)
            fa2_tiles.append(fa2_pair)

        # ---------------- edge side: tcb fp16 bcast tiles of (t+C) ----------------
        tcb = [[None] * NBLK for _ in range(NUM_HEADS)]
        for h in range(NUM_HEADS):
            for b in range(NBLK):
                sl = slice(h * N_EDGES + b * EBLK, h * N_EDGES + (b + 1) * EBLK)
                tC_row = prep.tile([1, EBLK], F32, tag="tC_row")
                nc.vector.tensor_scalar_add(tC_row[:, :], tsb[0:1, sl], C_OFF)
                ps = psum.tile([128, EBLK], F32, tag="ps")
                nc.tensor.matmul(ps[:, :], ones_row[:, :], tC_row[:, :],
                                 start=True, stop=True)
                t16 = persist.tile([128, EBLK], F16, tag=f"tcb{h}_{b}")
                nc.vector.tensor_copy(t16[:, :], ps[:, :])
                tcb[h][b] = t16

        # H fp16 staging DRAM (padded) for phase-C transposed reads
        h16_dram = dram.tile([NPAD, EPAD], F16)

        # ---------------- colsum (needs only fa tiles) ----------------
        csU = persist.tile([33, NUM_HEADS], F32)
        csU2 = persist.tile([33, NUM_HEADS], F32)
        for h in range(NUM_HEADS):
            ps_c = psA.tile([97, 1], F32, tag="psh0", name="ps_c")
            for k in range(NT):
                nc.tensor.matmul(ps_c[:, :], fa_tiles[k][:, h * 97:(h + 1) * 97],
                                 ones_col16[:, :], start=(k == 0), stop=(k == NT - 1))
            nc.vector.tensor_copy(csU[:, h:h + 1], ps_c[0:33, :])
            nc.vector.tensor_copy(csU2[:, h:h + 1], ps_c[64:97, :])
        half_csU = persist.tile([33, NUM_HEADS], F32)
        half_csU2 = persist.tile([33, NUM_HEADS], F32)
        nc.vector.tensor_scalar_mul(half_csU[:, :], csU[:, :], 0.5)
        nc.vector.tensor_scalar_mul(half_csU2[:, :], csU2[:, :], 0.5)

        # ---------------- phase A ----------------
        aggU = [persist.tile([33, N_EDGES], F32, tag=f"aggU{h}", name=f"aggU{h}")
                for h in range(NUM_HEADS)]

        for b in range(NBLK):
            e0 = b * EBLK
            ps_g = [psA.tile([97, EBLK], F32, tag=f"psg{h}", name=f"psg{h}")
                    for h in range(NUM_HEADS)]
            ps_h = [psA.tile([97, EBLK], F32, tag=f"psh{p}", name=f"psh{p}")
                    for p in range(2)]
            for k in range(NT):
                n0 = k * 128
                h16 = hpool.tile([128, EBLK], F16, tag="h16")
                for half in range(2):
                    plane = 2 * b + half
                    pu = hpool.tile([128, PBYTES], U8, tag="pu")
                    nc.vector.tensor_scalar(pu[:, :], pt_tiles[k][:, :], 7 - plane, 1,
                                            mybir.AluOpType.logical_shift_right,
                                            mybir.AluOpType.bitwise_and)
                    nc.vector.tensor_copy(h16[:, half * PBYTES:(half + 1) * PBYTES],
                                          pu[:, :])
                nc.sync.dma_start(h16_dram[n0:n0 + 128, e0:e0 + EBLK], h16[:, :])
                if b == 0:
                    nc.sync.dma_start(h16_dram[n0:n0 + 128, N_EDGES:EPAD], zpadc[:, :])
                first, last = (k == 0), (k == NT - 1)
                fa = fa_tiles[k]
                for h in range(NUM_HEADS):
                    htc = work.tile([128, EBLK], F16, tag="htc")
                    nc.vector.tensor_tensor(htc[:, :], h16[:, :], tcb[h][b][:, :],
                                            mybir.AluOpType.mult)
                    sgn = work.tile([128, EBLK], F16, tag="sgn")
                    nc.scalar.activation(sgn[:, :], htc[:, :],
                                         mybir.ActivationFunctionType.Sign,
                                         bias=sc_tiles[k][:, h:h + 1])
                    nc.tensor.matmul(ps_g[h][:, :], fa[:, h * 97:(h + 1) * 97],
                                     sgn[:, :], start=first, stop=last)
                for p in range(2):
                    nc.tensor.matmul(ps_h[p][:, :], fa2_tiles[k][p][:, :],
                                     h16[:, :], start=first, stop=last)
            # fused combine for this block, reading PSUM directly:
            #   A1u  = .5*psg[0:33]  + .5*csU ;  A1u2 = .5*psg[64:97] + .5*csU2
            #   aggU = v .* A1u + v2 .* (A2 - A1u2)
            for h in range(NUM_HEADS):
                p, hh = divmod(h, 2)
                sl = slice(e0, e0 + EBLK)
                tsl = slice(h * N_EDGES + e0, h * N_EDGES + e0 + EBLK)
                v_row = prep.tile([1, 2 * EBLK], F32, tag="v_row")
                nc.scalar.activation(v_row[:, 0:EBLK], tsb[0:1, tsl],
                                     mybir.ActivationFunctionType.Exp)
                nc.scalar.activation(v_row[:, EBLK:], tsb[0:1, tsl],
                                     mybir.ActivationFunctionType.Exp, scale=NEG_SLOPE)
                vb_ps = psum.tile([33, EBLK], F32, tag="ps")
                nc.tensor.matmul(vb_ps[:, :], ones_row[:, 0:33], v_row[:, 0:EBLK],
                                 start=True, stop=True)
                v2b_ps = psum.tile([33, EBLK], F32, tag="ps")
                nc.tensor.matmul(v2b_ps[:, :], ones_row[:, 0:33], v_row[:, EBLK:],
                                 start=True, stop=True)
                a1u = work.tile([33, EBLK], F32, tag="a1u")
                nc.vector.tensor_scalar(a1u[:, :], ps_g[h][0:33, :], 0.5,
                                        half_csU[:, h:h + 1], mybir.AluOpType.mult,
                                        mybir.AluOpType.add)
                a1u2 = work.tile([33, EBLK], F32, tag="a1u2")
                nc.vector.tensor_scalar(a1u2[:, :], ps_g[h][64:97, :], 0.5,
                                        half_csU2[:, h:h + 1], mybir.AluOpType.mult,
                                        mybir.AluOpType.add)
                d2 = work.tile([33, EBLK], F32, tag="d2")
                a2v = ps_h[p][0:33, :] if hh == 0 else ps_h[p][64:97, :]
                nc.vector.tensor_tensor(d2[:, :], a2v, a1u2[:, :],
                                        mybir.AluOpType.subtract)
                nc.vector.tensor_tensor(d2[:, :], d2[:, :], v2b_ps[:, :],
                                        mybir.AluOpType.mult)
                nc.vector.tensor_tensor(a1u[:, :], a1u[:, :], vb_ps[:, :],
                                        mybir.AluOpType.mult)
                nc.vector.tensor_tensor(aggU[h][:, sl], a1u[:, :], d2[:, :],
                                        mybir.AluOpType.add)

        # ---------------- collective ----------------
        cc_in = dram.tile([NUM_HEADS, 33, N_EDGES], F32)
        cc_out = dram.tile([NUM_HEADS, 33, N_EDGES], F32)
        for h in range(NUM_HEADS):
            nc.gpsimd.dma_start(cc_in[h, :, :], aggU[h][:, :])
        nc.gpsimd.collective_compute(
            "AllReduce",
            mybir.AluOpType.add,
            replica_groups=[list(range(CORES))],
            ins=[cc_in.opt()],
            outs=[cc_out.opt()],
        )
        for h in range(NUM_HEADS):
            nc.gpsimd.dma_start(aggU[h][:, :], cc_out[h, :, :])

        # ---------------- normalize -> hyper fp16 [128e, 128hd] x 16 ----------------
        hyper16 = [persist.tile([128, 128], F16, tag=f"hyp{et}", name=f"hyp{et}")
                   for et in range(ET)]
        for et in range(ET):
            e0 = et * 128
            ee = max(0, min(128, N_EDGES - e0))
            hyp = work.tile([128, 128], F32, tag="hyp")
            if ee < 128:
                nc.vector.memset(hyp[:], 0.0)
            for h in range(NUM_HEADS):
                if ee == 0:
                    continue
                tps = psum.tile([128, 33], F32, tag="ps")
                nc.tensor.transpose(tps[:ee, :], aggU[h][:, e0:e0 + ee],
                                    ident[0:33, 0:33])
                at = work.tile([128, 33], F32, tag="at")
                nc.vector.tensor_copy(at[:ee, :], tps[:ee, :])
                den = work.tile([128, 1], F32, tag="den")
                nc.vector.tensor_scalar_add(den[:ee, :], at[:ee, 32:33], 1e-9)
                rec = work.tile([128, 1], F32, tag="rec")
                nc.vector.reciprocal(rec[:ee, :], den[:ee, :])
                nc.vector.tensor_scalar_mul(hyp[:ee, h * 32:(h + 1) * 32],
                                            at[:ee, 0:32], rec[:ee, :])
            nc.vector.tensor_copy(hyper16[et][:, :], hyp[:, :])

        # ---------------- phase C: rst^T = hyper^T @ H^T ----------------
        # Stationary hyper[et] [128e,128hd] reused across all 5 node chunks;
        # one wide transposed DMA per e-tile; rst leaves transposed and the
        # host unscrambles (a cheap np transpose).
        rtags = [f"psg{j}" for j in range(4)] + ["psh0"]
        rps = [psA.tile([128, 512], F32, tag=rtags[c5], name=f"psr{c5}")
               for c5 in range(5)]
        for et in range(ET):
            htt = hpool.tile([128, NPAD], F16, tag="htt")
            nc.sync.dma_start_transpose(htt[:, :],
                                        h16_dram[0:NPAD, et * 128:(et + 1) * 128])
            for c5 in range(5):
                nc.tensor.matmul(rps[c5][:, :], hyper16[et][:, :],
                                 htt[:, c5 * 512:(c5 + 1) * 512],
                                 start=(et == 0), stop=(et == ET - 1))
        for c5 in range(5):
            n0 = c5 * 512
            nn = min(512, NPC - n0)
            rt = work.tile([128, 512], F16, tag="rt")
            nc.vector.tensor_copy(rt[:, :nn], rps[c5][:, :nn])
            nc.sync.dma_start(rstT_d[:, n0:n0 + nn], rt[:, :nn])

    return nc


PROFILE = False
LAST_RUN_NS = None

_CACHE = {}


def _get_nc():
    if "nc" not in _CACHE:
        nc = bacc.Bacc("TRN2", target_bir_lowering=False, debug=False,
                       enable_asserts=False, num_devices=CORES)
        build_kernel(nc)
        nc.compile()
        _CACHE["nc"] = nc
    return _CACHE["nc"]


def kernel(feat, edge_feat, H, fc_w, attn_src, attn_edge, src_idx=None, edge_idx=None,
           **extra):
    feat = np.asarray(feat, np.float32)
    edge_feat = np.asarray(edge_feat, np.float32)
    fc_w = np.asarray(fc_w, np.float32)
    a_src = np.asarray(attn_src, np.float32).reshape(NUM_HEADS, OUT_FEATS)
    a_edge = np.asarray(attn_edge, np.float32).reshape(NUM_HEADS, EDGE_DIM)

    # bit-packed incidence (big-endian bit order, matching np.packbits)
    if src_idx is not None and edge_idx is not None:
        si = np.asarray(src_idx, np.int64)
        ei = np.asarray(edge_idx, np.int64)
        hp = np.zeros((N_NODES, PBYTES), np.uint8)
        np.bitwise_or.at(hp, (si, ei >> 3),
                         np.right_shift(128, ei & 7).astype(np.uint8))
    else:
        hp = np.packbits(np.asarray(H, np.float32) != 0, axis=1)

    # t rows in bitplane-permuted edge order: col k*250+j <- edge 8j+k
    t = edge_feat @ a_edge.T                                   # [E, h]
    t_perm = np.ascontiguousarray(
        t.reshape(PBYTES, 8, NUM_HEADS).transpose(2, 1, 0).reshape(NUM_HEADS, N_EDGES)
    ).astype(np.float32)
    ws = (fc_w.reshape(IN_FEATS, NUM_HEADS, OUT_FEATS) * a_src[None]).sum(-1)

    feat16 = np.zeros((CORES, NPAD, IN_FEATS), np.float16)
    feat16[:, :NPC] = feat.reshape(CORES, NPC, IN_FEATS)
    hp8 = np.zeros((CORES, NPAD, PBYTES), np.uint8)
    hp8[:, :NPC] = hp.reshape(CORES, NPC, PBYTES)
    fcw16 = fc_w.astype(np.float16)
    ws16 = ws.astype(np.float16)

    nc = _get_nc()
    t_perm = t_perm.reshape(1, NUM_HEADS * N_EDGES)
    in_maps = [{
        "feat16": feat16[c],
        "hpack": hp8[c],
        "trow": t_perm,
        "fcw16": fcw16,
        "ws16": ws16,
    } for c in range(CORES)]
    import time as _time
    _t0 = _time.time()
    res = run_bass_kernel_spmd(nc, in_maps, list(range(CORES)))
    global LAST_RUN_NS
    LAST_RUN_NS = int((_time.time() - _t0) * 1e9)
    out = np.concatenate([res.results[c]["rstT"].T for c in range(CORES)], axis=0)
    return out.astype(np.float32)


# revision 18
# speedup vs baseline: 4.0748x; 1.1632x over previous
"""HGSA (hypergraph attention) layer on 8 trn2 NeuronCores.

Reference math:
  feat_src = (feat @ fc_w)  ->  [N, h, d]
  e(p)     = leaky_relu(s[src_p, h] + t[edge_p, h]);  s = feat_src . attn_src, t = edge_feat . attn_edge
  attn     = per-hyperedge softmax over incident pairs
  hyper[e] = seg_sum(attn * feat_src[src])            [E, h, d]
  rst      = H @ hyper                                [N, h*d]

Identities used (everything becomes dense matmuls over H; no dense exp/gather):
  - softmax max-subtraction cancels exactly; logits are O(1) so plain exp is safe.
  - exp(lrelu(x)), x = s+t, splits by sign r = [x>0]:
        w = r*u*v + (1-r)*u2*v2,  u=exp(s), v=exp(t), u2=exp(.2s), v2=exp(.2t)
  - with G1 = H .* r and Fu = [feat_src_h * u | u] (33 cols), Fu2 likewise:
        masked sums = v .* (Fu^T @ G1) + v2 .* (Fu2^T @ H - Fu2^T @ G1)
  - G1 is exact on-device: G1 = Relu(Sign(t_bcast + s)) .* H; a tie (s+t==0)
    gives 0, routing the pair to the u2*v2 branch where w is also exactly 1.

I/O diet (the axon tunnel, not the device, is the bottleneck):
  - H enters bit-packed (uint8, 8 edges/byte) and is unpacked on-device with
    vector shift/and into an fp16 0/1 tile. The edge axis is globally
    permuted into "bitplane" order e=8j+k -> k*250+j so the unpack writes
    contiguous 250-col blocks; t rows are permuted to match on the host and
    the permutation cancels everywhere else (it never leaves the edge axis).
  - feat enters fp16 and is loaded via transposed DMA (no on-chip transpose).
  - t = edge_feat . attn_edge and w_s = fc_w . attn_src are computed on the
    host (tiny) so edge_feat/attn_* never cross the wire.
  - rst leaves transposed as fp16; the host unscrambles with a np transpose.

Per-call dispatch cost also scales with instruction count, so the kernel is
structured for few, wide instructions: head-outer phase A over SBUF-resident
full-width H tiles (one Sign/Relu/mult per (head, node-tile) at 2000 edges
wide), stationary-operand reuse in the matmul loops, and a phase C that keeps
hyper[et] stationary against 512-node moving H^T panels.

Sharding: node rows split 2500/core (padded to 2560) over 8 cores; per-edge
aggregates AllReduce'd (two components: A1 sums f32, masked-u2 sums f16);
exp(t)/exp(.2t) weights are applied post-collective in the transposed domain
where they are per-partition scalars.

Layout note: SBUF/PSUM partition bases must be 0/32/64/96, so the per-head
stationary matrix is padded to 97 rows: [Fu (33) | zeros (31) | Fu2 (33)] and
extractions use bases 0 and 64.
"""

from contextlib import ExitStack

import numpy as np

import concourse.bass as bass
import concourse.mybir as mybir
import concourse.tile as tile
from concourse import bacc
from concourse.bass_utils import run_bass_kernel_spmd
from concourse.masks import make_identity

F32 = mybir.dt.float32
F16 = mybir.dt.float16
U8 = mybir.dt.uint8

N_NODES, N_EDGES = 20000, 2000
IN_FEATS, NUM_HEADS, OUT_FEATS, EDGE_DIM = 128, 4, 32, 64
NEG_SLOPE = 0.2
CORES = 8
NPC = N_NODES // CORES          # 2500 nodes per core
NPAD = 2560                     # padded nodes per core (20 full 128-tiles)
NT = NPAD // 128                # 20 node tiles per core
PBYTES = N_EDGES // 8           # 250 packed bytes per node row
EBLK = 500                      # PSUM-bank edge block = 2 bitplanes of 250
NBLK = N_EDGES // EBLK          # 4 edge blocks
EPAD = 2048                     # padded edges for H^T xbar loads
ET = EPAD // 128                # 16 e-tiles in dissemination


def build_kernel(nc):
    feat_d = nc.dram_tensor("feat16", [NPAD, IN_FEATS], F16, kind="ExternalInput").ap()
    hp_d = nc.dram_tensor("hpack", [NPAD, PBYTES], U8, kind="ExternalInput").ap()
    trow_d = nc.dram_tensor("trow", [1, NUM_HEADS * N_EDGES], F32, kind="ExternalInput").ap()
    fcw_d = nc.dram_tensor("fcw16", [IN_FEATS, IN_FEATS], F16, kind="ExternalInput").ap()
    ws_d = nc.dram_tensor("ws16", [IN_FEATS, NUM_HEADS], F16, kind="ExternalInput").ap()
    rstT_d = nc.dram_tensor("rstT", [NUM_HEADS * OUT_FEATS, NPC], F16, kind="ExternalOutput").ap()

    with tile.TileContext(nc) as tc, ExitStack() as ctx:
        consts = ctx.enter_context(tc.tile_pool(name="consts", bufs=1))
        persist = ctx.enter_context(tc.tile_pool(name="persist", bufs=1))
        work = ctx.enter_context(tc.tile_pool(name="work", bufs=2))
        dram = ctx.enter_context(tc.tile_pool(name="dram", bufs=1, space="DRAM"))

        ident = consts.tile([128, 128], F32)
        make_identity(nc, ident)
        ones_row = consts.tile([1, 128], F32)
        nc.gpsimd.memset(ones_row[:], 1.0)
        eps_col = consts.tile([128, 1], F32)
        nc.gpsimd.memset(eps_col[:], 1e-9)

        fcw = persist.tile([128, 128], F16)
        nc.sync.dma_start(fcw[:], fcw_d[:, :])
        wst = persist.tile([128, NUM_HEADS], F16)
        nc.sync.dma_start(wst[:], ws_d[:, :])

        h16_dram = dram.tile([NPAD, EPAD], F16)
        cc_in = dram.tile([NUM_HEADS, 33, N_EDGES], F32)

        fa_tiles, s_tiles, fa2_tiles, pt_tiles, h16_tiles = [], [], [], [], []
        tcb = []            # [128, N_EDGES] f16 bcast of t, per head
        vb, v2b = [], []    # [33, N_EDGES] f16 bcast of exp(t), exp(.2t), per head

        with tc.tile_pool(name="ptp", bufs=1) as ptp:

            with tc.tile_pool(name="prep", bufs=2) as prep, \
                 tc.tile_pool(name="edge", bufs=1) as edgep, \
                 tc.tile_pool(name="psum", bufs=2, space="PSUM") as psum:
                tsb = edgep.tile([1, NUM_HEADS * N_EDGES], F32, tag="tsb")
                nc.sync.dma_start(tsb[:], trow_d[:, :])

                # ---------------- node projections ----------------
                # fa[k]: [128, 4*97], head block = [Fu (33) | zeros (31) | Fu2 (33)]
                for k in range(NT):
                    n0 = k * 128
                    pt = ptp.tile([128, PBYTES], U8, tag=f"pt{k}", name=f"pt{k}")
                    nc.sync.dma_start(pt[:, :], hp_d[n0:n0 + 128, :])
                    pt_tiles.append(pt)
                    ftT = prep.tile([128, 128], F16, tag="ftT")
                    nc.sync.dma_start_transpose(ftT[:, :], feat_d[n0:n0 + 128, :])
                    fs_ps = psum.tile([128, 128], F32, tag="ps")
                    nc.tensor.matmul(fs_ps[:, :], ftT[:, :], fcw[:, :],
                                     start=True, stop=True)
                    s_ps = psum.tile([128, NUM_HEADS], F32, tag="pss")
                    nc.tensor.matmul(s_ps[:, :], ftT[:, :], wst[:, :],
                                     start=True, stop=True)
                    u_t = prep.tile([128, 2 * NUM_HEADS], F32, tag="u")
                    nc.scalar.activation(u_t[:, 0:NUM_HEADS], s_ps[:, :],
                                         mybir.ActivationFunctionType.Exp)
                    nc.scalar.activation(u_t[:, NUM_HEADS:], s_ps[:, :],
                                         mybir.ActivationFunctionType.Exp,
                                         scale=NEG_SLOPE)
                    s_col = persist.tile([128, NUM_HEADS], F32, tag=f"s{k}")
                    nc.vector.tensor_copy(s_col[:, :], s_ps[:, :])
                    s_tiles.append(s_col)

                    fa = persist.tile([128, NUM_HEADS * 97], F16, tag=f"fa{k}")
                    nc.vector.memset(fa[:], 0.0)
                    for h in range(NUM_HEADS):
                        u_c = u_t[:, h:h + 1]
                        u2_c = u_t[:, NUM_HEADS + h:NUM_HEADS + h + 1]
                        b0 = h * 97
                        nc.vector.tensor_scalar_mul(fa[:, b0:b0 + 32],
                                                    fs_ps[:, h * 32:(h + 1) * 32], u_c)
                        nc.vector.tensor_copy(fa[:, b0 + 32:b0 + 33], u_c)
                        nc.scalar.activation(fa[:, b0 + 64:b0 + 96],
                                             fs_ps[:, h * 32:(h + 1) * 32],
                                             mybir.ActivationFunctionType.Copy,
                                             scale=u2_c)
                        nc.scalar.copy(fa[:, b0 + 96:b0 + 97], u2_c)
                    fa_tiles.append(fa)
                    fa2_pair = []
                    for p in range(2):
                        fa2 = persist.tile([128, 97], F16, tag=f"fa2_{k}_{p}",
                                           name=f"fa2_{k}_{p}")
                        nc.vector.memset(fa2[:], 0.0)
                        h0, h1 = 2 * p, 2 * p + 1
                        nc.vector.tensor_copy(fa2[:, 0:33],
                                              fa[:, h0 * 97 + 64:h0 * 97 + 97])
                        nc.vector.tensor_copy(fa2[:, 64:97],
                                              fa[:, h1 * 97 + 64:h1 * 97 + 97])
                        fa2_pair.append(fa2)
                    fa2_tiles.append(fa2_pair)

                # ---------------- edge-side broadcast tiles ----------------
                for h in range(NUM_HEADS):
                    tcb_h = persist.tile([128, N_EDGES], F16, tag=f"tcb{h}",
                                         name=f"tcb{h}")
                    for b in range(NBLK):
                        sl = slice(h * N_EDGES + b * EBLK, h * N_EDGES + (b + 1) * EBLK)
                        ps = psum.tile([128, EBLK], F32, tag="pst")
                        nc.tensor.matmul(ps[:, :], ones_row[:, :], tsb[0:1, sl],
                                         start=True, stop=True)
                        nc.vector.tensor_copy(tcb_h[:, b * EBLK:(b + 1) * EBLK],
                                              ps[:, :])
                    tcb.append(tcb_h)
                    v_row = edgep.tile([1, 2 * N_EDGES], F32, tag="v_row")
                    tsl = slice(h * N_EDGES, (h + 1) * N_EDGES)
                    nc.scalar.activation(v_row[:, 0:N_EDGES], tsb[0:1, tsl],
                                         mybir.ActivationFunctionType.Exp)
                    nc.scalar.activation(v_row[:, N_EDGES:], tsb[0:1, tsl],
                                         mybir.ActivationFunctionType.Exp,
                                         scale=NEG_SLOPE)
                    vb_h = persist.tile([33, N_EDGES], F16, tag=f"vb{h}", name=f"vb{h}")
                    v2b_h = persist.tile([33, N_EDGES], F16, tag=f"v2b{h}",
                                         name=f"v2b{h}")
                    for b in range(NBLK):
                        bs = slice(b * EBLK, (b + 1) * EBLK)
                        ps = psum.tile([33, EBLK], F32, tag="psv", name="psv")
                        nc.tensor.matmul(ps[:, :], ones_row[:, 0:33],
                                         v_row[:, b * EBLK:(b + 1) * EBLK],
                                         start=True, stop=True)
                        nc.vector.tensor_copy(vb_h[:, bs], ps[:, :])
                        ps2 = psum.tile([33, EBLK], F32, tag="psv", name="psv2")
                        nc.tensor.matmul(ps2[:, :], ones_row[:, 0:33],
                                         v_row[:, N_EDGES + b * EBLK:N_EDGES + (b + 1) * EBLK],
                                         start=True, stop=True)
                        nc.vector.tensor_copy(v2b_h[:, bs], ps2[:, :])
                    vb.append(vb_h)
                    v2b.append(v2b_h)

            # ---------------- unpack H to resident fp16 tiles ----------------
            hp_ctx = ExitStack()
            hp = hp_ctx.enter_context(tc.tile_pool(name="hp", bufs=1))
            for k in range(NT):
                n0 = k * 128
                h16 = hp.tile([128, EPAD], F16, tag=f"h16_{k}", name=f"h16_{k}")
                nc.vector.memset(h16[:, N_EDGES:EPAD], 0.0)
                for plane in range(8):
                    pu = work.tile([128, PBYTES], U8, tag="pu")
                    nc.vector.tensor_scalar(pu[:, :], pt_tiles[k][:, :], 7 - plane, 1,
                                            mybir.AluOpType.logical_shift_right,
                                            mybir.AluOpType.bitwise_and)
                    nc.vector.tensor_copy(h16[:, plane * PBYTES:(plane + 1) * PBYTES],
                                          pu[:, :])
                nc.sync.dma_start(h16_dram[n0:n0 + 128, :], h16[:, :])
                h16_tiles.append(h16)

            # ---------------- phase A ----------------
            # For each head-pair p: A2 = fa2^T @ H (PSUM -> SBUF spill), then per
            # head: G1 = Relu(Sign(t + s)) .* H, A1 = fa^T @ G1, and the combine
            # z = vb .* A1u + v2b .* (A2 - A1u2) goes straight to the collective
            # staging buffer.
            a2sb = persist.tile([97, N_EDGES], F32)
            with tc.tile_pool(name="psA", bufs=1, space="PSUM") as psA:
                for p in range(2):
                    ps_b = [psA.tile([97, EBLK], F32, tag=f"psg{b}", name=f"psg{b}")
                            for b in range(NBLK)]
                    for k in range(NT):
                        for b in range(NBLK):
                            nc.tensor.matmul(ps_b[b][:, :], fa2_tiles[k][p][:, :],
                                             h16_tiles[k][:, b * EBLK:(b + 1) * EBLK],
                                             start=(k == 0), stop=(k == NT - 1))
                    for b in range(NBLK):
                        nc.vector.tensor_copy(a2sb[:, b * EBLK:(b + 1) * EBLK],
                                              ps_b[b][:, :])
                    for hh in range(2):
                        h = 2 * p + hh
                        r0 = 0 if hh == 0 else 64
                        ps_g = [psA.tile([97, EBLK], F32, tag=f"psg{b}", name=f"psh{b}")
                                for b in range(NBLK)]
                        for k in range(NT):
                            sgn = work.tile([128, N_EDGES], F16, tag="sgn")
                            nc.scalar.activation(sgn[:, :], tcb[h][:, :],
                                                 mybir.ActivationFunctionType.Sign,
                                                 bias=s_tiles[k][:, h:h + 1])
                            stp = work.tile([128, N_EDGES], F16, tag="stp")
                            nc.scalar.activation(stp[:, :], sgn[:, :],
                                                 mybir.ActivationFunctionType.Relu)
                            g1 = work.tile([128, N_EDGES], F16, tag="g1")
                            nc.vector.tensor_tensor(g1[:, :], stp[:, :],
                                                    h16_tiles[k][:, 0:N_EDGES],
                                                    mybir.AluOpType.mult)
                            for b in range(NBLK):
                                nc.tensor.matmul(ps_g[b][:, :],
                                                 fa_tiles[k][:, h * 97:(h + 1) * 97],
                                                 g1[:, b * EBLK:(b + 1) * EBLK],
                                                 start=(k == 0), stop=(k == NT - 1))
                        for b in range(NBLK):
                            bs = slice(b * EBLK, (b + 1) * EBLK)
                            d2 = work.tile([33, EBLK], F32, tag="d2")
                            nc.vector.tensor_tensor(d2[:, :], a2sb[r0:r0 + 33, bs],
                                                    ps_g[b][64:97, :],
                                                    mybir.AluOpType.subtract)
                            nc.vector.tensor_tensor(d2[:, :], d2[:, :], v2b[h][:, bs],
                                                    mybir.AluOpType.mult)
                            z = work.tile([33, EBLK], F32, tag="z")
                            nc.vector.tensor_tensor(z[:, :], ps_g[b][0:33, :],
                                                    vb[h][:, bs], mybir.AluOpType.mult)
                            zz = work.tile([33, EBLK], F32, tag="zz")
                            nc.vector.tensor_tensor(zz[:, :], z[:, :], d2[:, :],
                                                    mybir.AluOpType.add)
                            nc.sync.dma_start(cc_in[h, :, bs], zz[:, :])

            hp_ctx.close()

        # ---------------- collective ----------------
        cc_out = dram.tile([NUM_HEADS, 33, N_EDGES], F32)
        nc.gpsimd.collective_compute(
            "AllReduce",
            mybir.AluOpType.add,
            replica_groups=[list(range(CORES))],
            ins=[cc_in.opt()],
            outs=[cc_out.opt()],
        )

        # ---------------- normalize -> hyper fp16 [128e, 128hd] x 16 ----------------
        with tc.tile_pool(name="post", bufs=1) as post, \
             tc.tile_pool(name="psN", bufs=1, space="PSUM") as psN, \
             tc.tile_pool(name="psT", bufs=2, space="PSUM") as psT:
            agg = []
            for h in range(NUM_HEADS):
                agg_h = post.tile([33, N_EDGES], F32, tag=f"agg{h}", name=f"agg{h}")
                nc.sync.dma_start(agg_h[:, :], cc_out[h, :, :])
                agg.append(agg_h)
            hyper16 = []
            for et in range(ET):
                e0 = et * 128
                ee = max(0, min(128, N_EDGES - e0))
                hyp = work.tile([128, 128], F32, tag="hyp")
                if ee < 128:
                    nc.vector.memset(hyp[:], 0.0)
                for h in range(NUM_HEADS):
                    if ee == 0:
                        continue
                    tps = psT.tile([128, 33], F32, tag="tps")
                    nc.tensor.transpose(tps[:ee, :], agg[h][:, e0:e0 + ee],
                                        ident[0:33, 0:33])
                    den = work.tile([128, 1], F32, tag="den")
                    nc.vector.tensor_scalar_add(den[:ee, :], tps[:ee, 32:33], 1e-9)
                    rec = work.tile([128, 1], F32, tag="rec")
                    nc.vector.reciprocal(rec[:ee, :], den[:ee, :])
                    nc.vector.tensor_scalar_mul(hyp[:ee, h * 32:(h + 1) * 32],
                                                tps[:ee, 0:32], rec[:ee, :])
                h16t = post.tile([128, 128], F16, tag=f"hyp{et}", name=f"hyp{et}")
                nc.vector.tensor_copy(h16t[:, :], hyp[:, :])
                hyper16.append(h16t)

            # ---------------- phase C: rst^T = hyper^T @ H^T ----------------
            rps = [psN.tile([128, 512], F32, tag=f"pc{c5}", name=f"pc{c5}")
                   for c5 in range(5)]
            for et in range(ET):
                htt = post.tile([128, NPAD], F16, tag="htt")
                nc.sync.dma_start_transpose(htt[:, :],
                                            h16_dram[0:NPAD, et * 128:(et + 1) * 128])
                for c5 in range(5):
                    nc.tensor.matmul(rps[c5][:, :], hyper16[et][:, :],
                                     htt[:, c5 * 512:(c5 + 1) * 512],
                                     start=(et == 0), stop=(et == ET - 1))
            for c5 in range(5):
                n0 = c5 * 512
                nn = min(512, NPC - n0)
                rt = work.tile([128, 512], F16, tag="rt")
                nc.vector.tensor_copy(rt[:, :nn], rps[c5][:, :nn])
                nc.sync.dma_start(rstT_d[:, n0:n0 + nn], rt[:, :nn])

    return nc


PROFILE = False
LAST_RUN_NS = None

_CACHE = {}


def _get_nc():
    if "nc" not in _CACHE:
        nc = bacc.Bacc("TRN2", target_bir_lowering=False, debug=False,
                       enable_asserts=False, num_devices=CORES)
        build_kernel(nc)
        nc.compile()
        _CACHE["nc"] = nc
    return _CACHE["nc"]


def kernel(feat, edge_feat, H, fc_w, attn_src, attn_edge, src_idx=None, edge_idx=None,
           **extra):
    feat = np.asarray(feat, np.float32)
    edge_feat = np.asarray(edge_feat, np.float32)
    fc_w = np.asarray(fc_w, np.float32)
    a_src = np.asarray(attn_src, np.float32).reshape(NUM_HEADS, OUT_FEATS)
    a_edge = np.asarray(attn_edge, np.float32).reshape(NUM_HEADS, EDGE_DIM)

    # bit-packed incidence (big-endian bit order, matching np.packbits)
    if src_idx is not None and edge_idx is not None:
        si = np.asarray(src_idx, np.int64)
        ei = np.asarray(edge_idx, np.int64)
        hp = np.zeros((N_NODES, PBYTES), np.uint8)
        np.bitwise_or.at(hp, (si, ei >> 3),
                         np.right_shift(128, ei & 7).astype(np.uint8))
    else:
        hp = np.packbits(np.asarray(H, np.float32) != 0, axis=1)

    # t rows in bitplane-permuted edge order: col k*250+j <- edge 8j+k
    t = edge_feat @ a_edge.T                                   # [E, h]
    t_perm = np.ascontiguousarray(
        t.reshape(PBYTES, 8, NUM_HEADS).transpose(2, 1, 0).reshape(NUM_HEADS, N_EDGES)
    ).astype(np.float32).reshape(1, NUM_HEADS * N_EDGES)
    ws = (fc_w.reshape(IN_FEATS, NUM_HEADS, OUT_FEATS) * a_src[None]).sum(-1)

    feat16 = np.zeros((CORES, NPAD, IN_FEATS), np.float16)
    feat16[:, :NPC] = feat.reshape(CORES, NPC, IN_FEATS)
    hp8 = np.zeros((CORES, NPAD, PBYTES), np.uint8)
    hp8[:, :NPC] = hp.reshape(CORES, NPC, PBYTES)
    fcw16 = fc_w.astype(np.float16)
    ws16 = ws.astype(np.float16)

    nc = _get_nc()
    in_maps = [{
        "feat16": feat16[c],
        "hpack": hp8[c],
        "trow": t_perm,
        "fcw16": fcw16,
        "ws16": ws16,
    } for c in range(CORES)]
    import time as _time
    _t0 = _time.time()
    res = run_bass_kernel_spmd(nc, in_maps, list(range(CORES)))
    global LAST_RUN_NS
    LAST_RUN_NS = int((_time.time() - _t0) * 1e9)
    out = np.concatenate([res.results[c]["rstT"].T for c in range(CORES)], axis=0)
    return out.astype(np.float32)
